# revision 15
# baseline (speedup 1.0000x reference)
"""Cross-attention kernel for Trainium2, sharded over 8 NeuronCores.

Problem (per reference):
  q = wq @ x_q + bq ; k = wk @ x_kv + bk ; v = wv @ x_kv + bv   (1x1 convs)
  per head: attn = softmax(q^T k / sqrt(hd)) ; out = attn @ v^T
  y = wo @ out + bo

Sharding: core c -> (batch b = c // 4, head n = c % 4). Each core runs one
head's full attention and produces the partial output projection
y_part = wo[:, head] @ out_head; the host sums the 4 head partials per batch.

Device-side simplifications (all mathematically exact):
  * bk drops out entirely (per-query constant shift cancels in softmax).
  * bv folds into the output bias on the host (sum_j softmax_ij = 1).
  * scale 1/8 folds into wq/bq on the host.
  * exp computes e^(s - 2.5): the constant shift cancels in the deferred
    host-side normalization and keeps e^s under fp8-e4m3's 448 max.
  * softmax denominator comes from a ones-column appended to v^T in the AV
    matmul.
  * normalization is deferred past the output projection to the host:
    the device ships y_un (bf16) plus per-pixel denominators (f32); the
    host computes y_un / den + bias.

Performance structure (from NTFF traces): the loop is bound by the scalar
engine's exp ([128,1024] PSUM->SBUF sustains ~1.07us back-to-back), but the
chip enforces an activity power cap: sustained PE-array duty above ~80% of
2.4GHz trips a 50%-utilization throttle (observed: 91% duty -> throttled,
77% -> clean). The kernel therefore (a) software-pipelines QK(j) -> exp(j)
-> AV(j-2..j-3) with the logit tile st triple-buffered in PSUM (3x2 banks
+ 2 for the AV accumulator = all 8) so consecutive exps never have PE work
on their dependency path, and (b) runs HALF the AV block-pairs as fp8-e4m3
DoubleRow matmuls (two j-blocks contracted per pass, halving those AV
cycles) to hold PE duty at ~77%, under the cap. fp8 on post-softmax
weights/values is accuracy-safe (~1.4e-2 total rel err vs the 2e-2 gate).
The scalar engine runs nothing but exps; inputs stream in 512-col chunks
(gpsimd ring: weights + x_kv + back-half x_q; sync ring: front x_q, the
four v^T chunk transposes, y outputs) with projections and transposes
interleaved into the first i-chunk."""

import numpy as np
import ml_dtypes

import concourse.bacc as bacc
import concourse.mybir as mybir
import concourse.tile as tile
from concourse.bass_utils import run_bass_kernel_spmd

F32 = mybir.dt.float32
BF16 = mybir.dt.bfloat16
F8 = mybir.dt.float8e4

B, C, HGT, WID = 2, 256, 64, 64
S = HGT * WID  # 4096 pixels
NH, HD = 4, 64
NCORES = 8
P = 128
IC = 1024  # i-chunk width (2 PSUM banks)
NI = S // IC  # 4
NJ = S // P  # 32 j-blocks
SCALE = HD ** -0.5
EXP_SHIFT = 2.5  # exp(s - shift): cancels in host normalization


def _fp8_pair(p):
    """Which global block-pairs run the AV in fp8 DoubleRow."""
    return p % 2 == 0


def _emit(tc):
    nc = tc.nc
    xq = nc.dram_tensor("xq", [2, P, S], BF16, kind="ExternalInput").ap()
    xkv = nc.dram_tensor("xkv", [2, P, S], BF16, kind="ExternalInput").ap()
    wqT = nc.dram_tensor("wqT", [2, P, HD], BF16, kind="ExternalInput").ap()
    wkvT = nc.dram_tensor("wkvT", [2, P, P], BF16, kind="ExternalInput").ap()
    woT = nc.dram_tensor("woT", [HD, C], BF16, kind="ExternalInput").ap()
    bq = nc.dram_tensor("bq", [HD, 1], F32, kind="ExternalInput").ap()
    gate = nc.dram_tensor("gate", [P, 1], F32, kind="ExternalInput").ap()
    y = nc.dram_tensor("y", [2, P, S], BF16, kind="ExternalOutput").ap()
    yden = nc.dram_tensor("yden", [1, S], F32, kind="ExternalOutput").ap()

    with (
        tc.tile_pool(name="const", bufs=1) as cpool,
        tc.tile_pool(name="xp", bufs=1) as xpool,
        tc.tile_pool(name="qkv", bufs=1) as qpool,
        tc.tile_pool(name="es", bufs=1) as epool,
        tc.tile_pool(name="epi", bufs=2) as fpool,
        tc.tile_pool(name="ps", bufs=1, space="PSUM") as pp,
    ):
        # ---- exp bias + table load ASAP (first DVE/ACT work) ----
        ebias_sb = cpool.tile([P, 1], F32)
        nc.vector.memset(ebias_sb[:], -EXP_SHIFT)
        warm_sb = cpool.tile([P, 1], BF16)
        nc.scalar.activation(warm_sb[:], ebias_sb[:],
                             mybir.ActivationFunctionType.Exp,
                             bias=ebias_sb[:])

        # ---- weights + x_kv chunks on the gpsimd (SWDGE) ring ----
        wq_sb = cpool.tile([P, 2 * HD], BF16)
        wkv_sb = cpool.tile([P, 2 * P], BF16)
        wo_sb = cpool.tile([HD, C], BF16)
        bq_sb = cpool.tile([HD, 1], F32)
        xq_sb = [xpool.tile([P, S], BF16, tag=f"xq{i}", name=f"xq_sb{i}")
                 for i in range(2)]
        xkv_sb = [xpool.tile([P, S], BF16, tag=f"xkv{i}", name=f"xkv_sb{i}")
                  for i in range(2)]

        KCH = 512  # x_kv arrival chunk (one kv projection's worth)

        def xkv_chunk(c):
            sl = slice(c * KCH, (c + 1) * KCH)
            nc.gpsimd.dma_start(xkv_sb[0][:, sl], xkv[0][:, sl])
            nc.gpsimd.dma_start(xkv_sb[1][:, sl], xkv[1][:, sl])

        def xq_quarter(eng, qt):
            sl = slice(qt * IC, (qt + 1) * IC)
            eng.dma_start(xq_sb[0][:, sl], xq[0][:, sl])
            eng.dma_start(xq_sb[1][:, sl], xq[1][:, sl])

        # The "gate" DMA (a [P,1] zero vector) is issued late on the gpsimd
        # ring; the fp8 cast of va group 0 adds it (+0.0, exact), so the
        # first AV pair -- and with it the whole in-order PE stream past
        # the first four QK/exp blocks -- waits for it. That engineers a
        # ~3us all-engines-quiet window at ~10us, which is what the power
        # manager needs to grant the 2.4GHz PE clock before the main loop.
        gate_sb = cpool.tile([P, 1], F32)
        nc.gpsimd.dma_start(wkv_sb[:, 0:P], wkvT[0])
        nc.gpsimd.dma_start(wkv_sb[:, P:2 * P], wkvT[1])
        xkv_chunk(0)
        nc.gpsimd.dma_start(wq_sb[:, 0:HD], wqT[0])
        nc.gpsimd.dma_start(wq_sb[:, HD:2 * HD], wqT[1])
        nc.gpsimd.dma_start(bq_sb[:], bq)
        xkv_chunk(1)
        nc.gpsimd.dma_start(wo_sb[:], woT)
        for c in range(2, 8):
            xkv_chunk(c)
        nc.gpsimd.dma_start(gate_sb[:], gate)
        xq_quarter(nc.gpsimd, 2)
        xq_quarter(nc.gpsimd, 3)

        # x_q front half on the sync ring (cheap issues, early data); the
        # sync ring then carries only the v^T transposes + y outputs, so
        # the xbar-mode switch never waits behind bulk copy DMAs.
        xq_quarter(nc.sync, 0)
        xq_quarter(nc.sync, 1)

        # ---- persistent SBUF tensors ----
        # q/k kept at their true 64 partitions: matmul contracts over 64.
        q_sb = qpool.tile([HD, S], BF16)
        k_sb = qpool.tile([HD, S], BF16)
        v_sb = qpool.tile([P, S], BF16)  # rows 64:128 hold v
        # v^T blocks: [j-part, (block, 128)]; cols 0:64 = v^T (transpose
        # target), col 64 = ones (denominator row), rest zero. va8 is the
        # fp8 copy used by the DoubleRow AV pairs.
        va_sb = qpool.tile([P, NJ * P], BF16)
        va_v = va_sb.rearrange("p (j c) -> p j c", c=P)
        va8_sb = qpool.tile([P, NJ * P], F8)
        va8_v = va8_sb.rearrange("p (j c) -> p j c", c=P)
        nc.vector.memset(va_sb[:], 0.0)
        nc.vector.memset(va_v[:, :, HD:HD + 1], 1.0)

        # Projection pieces. In-loop projections are emitted one matmul at
        # a time (with an ACT pacing bubble on those blocks) so the
        # post-grant PE duty never spikes past the chip's activity power
        # cap (~77% sustained trips a 50% throttle).
        proj_psum = {}

        def kv_proj_mult(t, half):
            sl = slice(t * 512, (t + 1) * 512)
            if half == 0:
                proj_psum[("kv", t)] = pp.tile([P, 512], F32, tag="s",
                                               bufs=3, name="kvp")
                nc.tensor.matmul(proj_psum[("kv", t)][:], wkv_sb[:, 0:P],
                                 xkv_sb[0][:, sl], start=True, stop=False)
            else:
                kvp = proj_psum.pop(("kv", t))
                nc.tensor.matmul(kvp[:], wkv_sb[:, P:2 * P],
                                 xkv_sb[1][:, sl], start=False, stop=True)
                nc.vector.tensor_copy(k_sb[:, sl], kvp[0:HD, :])
                nc.vector.tensor_copy(v_sb[HD:P, sl], kvp[HD:P, :])

        def q_proj_mult(t, half):
            sl = slice(t * 512, (t + 1) * 512)
            if half == 0:
                proj_psum[("q", t)] = pp.tile([HD, 512], F32, tag="s",
                                              bufs=3, name="qp")
                nc.tensor.matmul(proj_psum[("q", t)][:], wq_sb[:, 0:HD],
                                 xq_sb[0][:, sl], start=True, stop=False)
            else:
                qp = proj_psum.pop(("q", t))
                nc.tensor.matmul(qp[:], wq_sb[:, HD:2 * HD],
                                 xq_sb[1][:, sl], start=False, stop=True)
                nc.vector.tensor_scalar_add(q_sb[:, sl], qp[:], bq_sb[:])

        def kv_proj(t):
            kv_proj_mult(t, 0)
            kv_proj_mult(t, 1)

        def q_proj(t):
            q_proj_mult(t, 0)
            q_proj_mult(t, 1)

        def v_transpose(g):  # v cols [1024g, 1024g+1024) -> va blocks 8g..
            nc.sync.dma_start_transpose(
                out=va_v[:, 8 * g:8 * (g + 1), 0:HD],
                in_=v_sb[HD:P, g * IC:(g + 1) * IC])

        def va_cast(g, gated=False):
            # fp8 copy for the DoubleRow pairs (picks up ones column too);
            # group 0 adds the gate zeros (exact) to stall the PE stream
            # until the gate DMA lands.
            gsl = slice(8 * g * P, 8 * (g + 1) * P)
            if gated:
                nc.vector.tensor_scalar_add(va8_sb[:, gsl], va_sb[:, gsl],
                                            gate_sb[:])
            else:
                nc.vector.tensor_copy(va8_sb[:, gsl], va_sb[:, gsl])

        # ACT pacing bubble: a near-empty activation stretching the exp
        # period to hold the sustained energy rate at the power cap
        # (~1.27us/block, the proven-sustainable pace). Negligible energy.
        pace_sb = cpool.tile([P, 1], BF16)

        def act_pace():
            nc.scalar.activation(pace_sb[:], va_sb[:, 0:1],
                                 mybir.ActivationFunctionType.Exp,
                                 bias=ebias_sb[:])

        # Interleave schedule: (i, j) -> work emitted before QK(i, j).
        # Transposes/casts are EMITTED before their first consumer AV
        # (otherwise the framework orders the write after those reads,
        # which would then see the memset zeros). The front of the input
        # (kv chunks 0-3, q chunks 0-1, transposes 0-1) is emitted before
        # the loop; the rest is metered one matmul per block.
        pre = {}

        def sched(i, j, fn, pace=False):
            pre.setdefault((i, j), []).append((fn, pace))

        for c in range(4, 8):
            jj = 4 * c - 9  # 7, 11, 15, 19 (block 4c first needs chunk c)
            sched(0, jj, lambda c=c: kv_proj_mult(c, 0))
            sched(0, jj + 1, lambda c=c: kv_proj_mult(c, 1))
        sched(0, 13, lambda: v_transpose(2))
        sched(0, 14, lambda: va_cast(2))
        sched(0, 21, lambda: v_transpose(3))
        sched(0, 22, lambda: va_cast(3))
        sched(0, 24, lambda: q_proj_mult(2, 0))
        sched(0, 25, lambda: q_proj_mult(2, 1))
        sched(0, 26, lambda: q_proj_mult(3, 0))
        sched(0, 27, lambda: q_proj_mult(3, 1))
        for i, t0 in ((1, 4), (2, 6)):
            for dt in range(2):
                sched(i, 12 + 4 * dt, lambda t=t0 + dt: q_proj_mult(t, 0))
                sched(i, 13 + 4 * dt, lambda t=t0 + dt: q_proj_mult(t, 1))

        # ---- epilogue pieces ----
        pend = [None] * NI  # per chunk: unnormalized out^T awaiting out-proj

        def drain(i, av):
            # move the (unnormalized) attention output + denominators out
            # of PSUM so the av banks free up; DVE only.
            outt = fpool.tile([HD, IC], BF16, name="outt")
            nc.vector.tensor_copy(outt[:], av[0:HD, :])
            den = fpool.tile([1, IC], F32, name="den")
            nc.vector.tensor_copy(den[:], av[HD:HD + 1, :])
            nc.gpsimd.dma_start(yden[:, i * IC:(i + 1) * IC], den[:])
            pend[i] = outt

        def out_proj_step(i, s, final=False):
            # one quarter of chunk i's output projection
            outt = pend[i]
            oh, h = divmod(s, 2)
            yp = pp.tile([P, 512], F32, tag="s", bufs=3, name="yp")
            nc.tensor.matmul(yp[:], wo_sb[:, oh * P:(oh + 1) * P],
                             outt[:, h * 512:(h + 1) * 512],
                             start=True, stop=True)
            ys = fpool.tile([P, 512], BF16, name="ys", tag="ys", bufs=4)
            if final and s >= 2:
                # ACT is idle after the last exp: split the drains
                nc.scalar.activation(ys[:], yp[:],
                                     mybir.ActivationFunctionType.Copy)
            else:
                nc.vector.tensor_copy(ys[:], yp[:])
            nc.sync.dma_start(
                y[oh][:, i * IC + h * 512:i * IC + (h + 1) * 512], ys[:])

        # ---- the attention loop ----
        # Per global block b = i*NJ + j: QK(b) -> exp(b) -> AV(b-3, b-2)
        # at odd b. st triple-buffered: back-to-back exps never wait on PE.
        av_tiles = [None] * NI
        et8 = {}   # pair index -> [P, 2*IC] fp8 tile
        et16 = {}  # block index -> [P, IC] bf16 tile

        def av_pair(b0):  # blocks b0, b0+1 (same chunk: chunks 32-aligned)
            i = b0 // NJ
            j0 = b0 % NJ
            p = b0 // 2
            if av_tiles[i] is None:
                av_tiles[i] = pp.tile([P, IC], F32, tag="av", bufs=1,
                                      name="av")
            av = av_tiles[i]
            start = j0 == 0
            stop = j0 == NJ - 2
            # start/stop are per accumulation REGION (each 512-col bank)
            if _fp8_pair(p):
                e8 = et8.pop(p)
                e8v = e8.rearrange("p (t i) -> p t i", t=2)
                for h in range(2):
                    nc.tensor.matmul(
                        av[:, h * 512:(h + 1) * 512],
                        va8_v[:, j0:j0 + 2, :],
                        e8v[:, :, h * 512:(h + 1) * 512],
                        start=start, stop=stop,
                        perf_mode=mybir.MatmulPerfMode.DoubleRow)
            else:
                for jj in (b0, b0 + 1):
                    et = et16.pop(jj)
                    for h in range(2):
                        nc.tensor.matmul(
                            av[:, h * 512:(h + 1) * 512],
                            va_v[:, jj % NJ, :],
                            et[:, h * 512:(h + 1) * 512],
                            start=start and jj == b0,
                            stop=stop and jj == b0 + 1)
            if stop:
                drain(i, av)
                av_tiles[i] = None

        # ---- pre-loop head: front projections at the cold clock ----
        kv_proj(0)
        kv_proj(1)
        q_proj(0)
        q_proj(1)
        v_transpose(0)
        kv_proj(2)
        kv_proj(3)
        v_transpose(1)
        va_cast(0, gated=True)
        va_cast(1)

        NB = NI * NJ
        for b in range(NB):
            i, j = divmod(b, NJ)
            pace = b % 2 == 1  # hold the sustained rate at the power cap
            for fn, pc in pre.get((i, j), ()):
                fn()
                pace = pace or pc
            if i > 0 and 2 <= j <= 8 and j % 2 == 0:
                # one quarter of the previous chunk's output projection,
                # spread to stay under the power cap
                out_proj_step(i - 1, j // 2 - 1)
            # QK + exp for block b
            jb = slice(j * P, (j + 1) * P)
            st = pp.tile([P, IC], F32, tag="s", bufs=3, name="st")
            for h in range(2):
                isl = slice(i * IC + h * 512, i * IC + (h + 1) * 512)
                nc.tensor.matmul(st[:, h * 512:(h + 1) * 512],
                                 k_sb[:, jb], q_sb[:, isl],
                                 start=True, stop=True)
            p = b // 2
            if _fp8_pair(p):
                if b % 2 == 0:
                    et8[p] = epool.tile([P, 2 * IC], F8, name="et8",
                                        tag="et8", bufs=3)
                nc.scalar.activation(et8[p][:, (b % 2) * IC:(b % 2 + 1) * IC],
                                     st[:],
                                     mybir.ActivationFunctionType.Exp,
                                     bias=ebias_sb[:])
            else:
                et16[b] = epool.tile([P, IC], BF16, name="et16",
                                     tag="et16", bufs=6)
                nc.scalar.activation(et16[b][:], st[:],
                                     mybir.ActivationFunctionType.Exp,
                                     bias=ebias_sb[:])
            if pace:
                act_pace()
            if b >= 3 and b % 2 == 1:
                av_pair(b - 3)
        av_pair(NB - 2)
        for s in range(4):
            out_proj_step(NI - 1, s, final=True)


def build():
    nc = bacc.Bacc("TRN2", target_bir_lowering=False, debug=False,
                   enable_asserts=False)
    with tile.TileContext(nc) as tc:
        _emit(tc)
    nc.compile()
    return nc


_NC_CACHE = []


def _get_nc():
    if not _NC_CACHE:
        _NC_CACHE.append(build())
    return _NC_CACHE[0]


def make_in_maps(x_q, x_kv, wq, bq, wk, bk, wv, bv, wo, bo):
    bf = ml_dtypes.bfloat16
    in_maps = []
    bo_effs = []
    for c in range(NCORES):
        b, n = divmod(c, NH)
        hs = slice(n * HD, (n + 1) * HD)
        wq_h = wq[hs].astype(np.float64) * SCALE
        bo_eff = wo[:, hs].astype(np.float64) @ bv[hs].astype(np.float64)
        if n == 0:
            bo_eff = bo_eff + bo.astype(np.float64)
        bo_effs.append(bo_eff.astype(np.float32))
        in_maps.append({
            "xq": np.ascontiguousarray(
                x_q[b].reshape(C, S).reshape(2, P, S)).astype(bf),
            "xkv": np.ascontiguousarray(
                x_kv[b].reshape(C, S).reshape(2, P, S)).astype(bf),
            "wqT": np.ascontiguousarray(wq_h.T.reshape(2, P, HD)).astype(bf),
            "wkvT": np.ascontiguousarray(
                np.concatenate([wk[hs].T, wv[hs].T], axis=1)
                .reshape(2, P, P)).astype(bf),
            "woT": np.ascontiguousarray(wo[:, hs].T).astype(bf),
            "bq": (bq[hs].astype(np.float64) * SCALE
                   ).astype(np.float32).reshape(HD, 1),
            "gate": np.zeros((P, 1), np.float32),
        })
    return in_maps, bo_effs


def assemble_output(results, bo_effs):
    # y_core is the unnormalized head partial (bf16, scaled by e^-shift);
    # divide by the (identically scaled) denominator and add the
    # host-folded bias here.
    y = np.zeros((B, C, S), np.float32)
    for c in range(NCORES):
        b = c // NH
        den = results[c]["yden"].reshape(1, S).astype(np.float32)
        y[b] += results[c]["y"].astype(np.float32).reshape(C, S) / den \
            + bo_effs[c].reshape(C, 1)
    return y.reshape(B, C, HGT, WID)


def kernel(**inputs):
    nc = _get_nc()
    in_maps, bo_effs = make_in_maps(**inputs)
    res = run_bass_kernel_spmd(nc, in_maps, list(range(NCORES)))
    return assemble_output(res.results, bo_effs)


if __name__ == "__main__":
    nc = build()
    print("built + compiled ok")


# revision 16
# speedup vs baseline: 1.0306x; 1.0306x over previous
"""Cross-attention kernel for Trainium2, sharded over 8 NeuronCores.

Problem (per reference):
  q = wq @ x_q + bq ; k = wk @ x_kv + bk ; v = wv @ x_kv + bv   (1x1 convs)
  per head: attn = softmax(q^T k / sqrt(hd)) ; out = attn @ v^T
  y = wo @ out + bo

Sharding: core c -> (batch b = c // 4, head n = c % 4). Each core runs one
head's full attention and produces the partial output projection
y_part = wo[:, head] @ out_head; the host sums the 4 head partials per batch.

Device-side simplifications (all mathematically exact):
  * bk drops out entirely (per-query constant shift cancels in softmax).
  * bv folds into the output bias on the host (sum_j softmax_ij = 1).
  * scale 1/8 folds into wq/bq on the host.
  * exp computes e^(s - 2.5); the shift cancels in the deferred host-side
    normalization (headroom only).
  * softmax denominator comes from a ones-column appended to v^T in the AV
    matmul.
  * normalization is deferred past the output projection to the host: the
    device ships y_un (bf16) plus per-pixel denominators (f32); the host
    computes y_un / den + bias.

Performance model (from NTFF traces of many variants): the chip enforces an
activity power cap evaluated per 3413ns (4096-cycle) window. While the PE
holds its 2.4GHz clock grant, windows whose scalar-engine busy fraction is
~always-on get the grant revoked (PE forced to 50% util for 10+ windows);
pure PE-matmul bursts pass. The sustainable operating point is the one this
loop is built around: exp(j) [1.11us on ACT] with the AV(j) + QK(j+2)
matmuls on its dependency path via a 2-deep PSUM logit buffer, which
self-paces ACT at ~88% busy. Before the grant (cold 1.2GHz PE), ACT runs
exp at 2GHz (683ns) and no cap applies - so the head BANKS as many QK+exp
blocks as possible at the cold-clock pace, then a gate DMA holds the first
AV back to give the power manager the ~3us quiet window it needs to grant
2.4GHz, after which the banked AVs drain inside the loop's ACT-idle gaps.
Inputs stream in 512-col chunks (gpsimd ring: weights + x_kv + gate;
sync ring: front x_q, chunked v^T transposes, y outputs); k/v/q
projections are metered one matmul per block around the banked phase."""

import numpy as np
import ml_dtypes

import concourse.bacc as bacc
import concourse.mybir as mybir
import concourse.tile as tile
from concourse.bass_utils import run_bass_kernel_spmd

F32 = mybir.dt.float32
BF16 = mybir.dt.bfloat16

B, C, HGT, WID = 2, 256, 64, 64
S = HGT * WID  # 4096 pixels
NH, HD = 4, 64
NCORES = 8
P = 128
IC = 1024  # i-chunk width (2 PSUM banks)
NI = S // IC  # 4
NJ = S // P  # 32 j-blocks
SCALE = HD ** -0.5
EXP_SHIFT = 2.5
PRE = 10  # blocks banked at the cold clock before the grant gate


def _emit(tc):
    nc = tc.nc
    xq = nc.dram_tensor("xq", [2, P, S], BF16, kind="ExternalInput").ap()
    xkv = nc.dram_tensor("xkv", [2, P, S], BF16, kind="ExternalInput").ap()
    wqT = nc.dram_tensor("wqT", [2, P, HD], BF16, kind="ExternalInput").ap()
    wkvT = nc.dram_tensor("wkvT", [2, P, P], BF16, kind="ExternalInput").ap()
    woT = nc.dram_tensor("woT", [HD, C], BF16, kind="ExternalInput").ap()
    bq = nc.dram_tensor("bq", [HD, 1], F32, kind="ExternalInput").ap()
    gate = nc.dram_tensor("gate", [P, 1], F32, kind="ExternalInput").ap()
    y = nc.dram_tensor("y", [2, P, S], BF16, kind="ExternalOutput").ap()
    yden = nc.dram_tensor("yden", [1, S], F32, kind="ExternalOutput").ap()

    with (
        tc.tile_pool(name="const", bufs=1) as cpool,
        tc.tile_pool(name="xp", bufs=1) as xpool,
        tc.tile_pool(name="qkv", bufs=1) as qpool,
        tc.tile_pool(name="es", bufs=PRE + 3) as epool,
        tc.tile_pool(name="epi", bufs=2) as fpool,
        tc.tile_pool(name="ps", bufs=2, space="PSUM") as pp,
    ):
        # ---- exp bias + table load ASAP (first DVE/ACT work) ----
        ebias_sb = cpool.tile([P, 1], F32)
        nc.vector.memset(ebias_sb[:], -EXP_SHIFT)
        warm_sb = cpool.tile([P, 1], BF16)
        nc.scalar.activation(warm_sb[:], ebias_sb[:],
                             mybir.ActivationFunctionType.Exp,
                             bias=ebias_sb[:])

        # ---- weights + x_kv chunks + gate on the gpsimd (SWDGE) ring ----
        wq_sb = cpool.tile([P, 2 * HD], BF16)
        wkv_sb = cpool.tile([P, 2 * P], BF16)
        wo_sb = cpool.tile([HD, C], BF16)
        bq_sb = cpool.tile([HD, 1], F32)
        gate_sb = cpool.tile([P, 1], F32)
        dly_sb = cpool.tile([P, IC], BF16)  # delay ballast for the gate
        xq_sb = [xpool.tile([P, S], BF16, tag=f"xq{i}", name=f"xq_sb{i}")
                 for i in range(2)]
        xkv_sb = [xpool.tile([P, S], BF16, tag=f"xkv{i}", name=f"xkv_sb{i}")
                  for i in range(2)]

        KCH = 512

        def xkv_chunk(c):
            sl = slice(c * KCH, (c + 1) * KCH)
            nc.gpsimd.dma_start(xkv_sb[0][:, sl], xkv[0][:, sl])
            nc.gpsimd.dma_start(xkv_sb[1][:, sl], xkv[1][:, sl])

        def xq_quarter(eng, qt):
            sl = slice(qt * IC, (qt + 1) * IC)
            eng.dma_start(xq_sb[0][:, sl], xq[0][:, sl])
            eng.dma_start(xq_sb[1][:, sl], xq[1][:, sl])

        nc.gpsimd.dma_start(wkv_sb[:, 0:P], wkvT[0])
        nc.gpsimd.dma_start(wkv_sb[:, P:2 * P], wkvT[1])
        xkv_chunk(0)
        nc.gpsimd.dma_start(wq_sb[:, 0:HD], wqT[0])
        nc.gpsimd.dma_start(wq_sb[:, HD:2 * HD], wqT[1])
        nc.gpsimd.dma_start(bq_sb[:], bq)
        xkv_chunk(1)
        nc.gpsimd.dma_start(wo_sb[:], woT)
        for c in range(2, 8):
            xkv_chunk(c)
        # delay ballast then the gate: the gate lands once the input stream
        # has drained, timing the quiet window right after the banked phase
        nc.gpsimd.dma_start(dly_sb[:], xkv[0][:, 0:IC])
        nc.gpsimd.dma_start(gate_sb[:], gate)
        xq_quarter(nc.gpsimd, 2)
        xq_quarter(nc.gpsimd, 3)

        # x_q front half on the sync ring (cheap issues, early data); the
        # sync ring then carries the v^T transposes + y outputs only.
        xq_quarter(nc.sync, 0)
        xq_quarter(nc.sync, 1)

        # ---- persistent SBUF tensors ----
        q_sb = qpool.tile([HD, S], BF16)
        k_sb = qpool.tile([HD, S], BF16)
        v_sb = qpool.tile([P, S], BF16)  # rows 64:128 hold v
        va_sb = qpool.tile([P, NJ * P], BF16)
        va_v = va_sb.rearrange("p (j c) -> p j c", c=P)
        nc.vector.memset(va_sb[:], 0.0)
        nc.vector.memset(va_v[:, :, HD:HD + 1], 1.0)

        proj_psum = {}

        def kv_proj_mult(t, half):
            sl = slice(t * 512, (t + 1) * 512)
            if half == 0:
                proj_psum[("kv", t)] = pp.tile([P, 512], F32, tag="s",
                                               bufs=2, name="kvp")
                nc.tensor.matmul(proj_psum[("kv", t)][:], wkv_sb[:, 0:P],
                                 xkv_sb[0][:, sl], start=True, stop=False)
            else:
                kvp = proj_psum.pop(("kv", t))
                nc.tensor.matmul(kvp[:], wkv_sb[:, P:2 * P],
                                 xkv_sb[1][:, sl], start=False, stop=True)
                nc.vector.tensor_copy(k_sb[:, sl], kvp[0:HD, :])
                nc.vector.tensor_copy(v_sb[HD:P, sl], kvp[HD:P, :])

        def q_proj_mult(t, half):
            sl = slice(t * 512, (t + 1) * 512)
            if half == 0:
                proj_psum[("q", t)] = pp.tile([HD, 512], F32, tag="s",
                                              bufs=2, name="qp")
                nc.tensor.matmul(proj_psum[("q", t)][:], wq_sb[:, 0:HD],
                                 xq_sb[0][:, sl], start=True, stop=False)
            else:
                qp = proj_psum.pop(("q", t))
                nc.tensor.matmul(qp[:], wq_sb[:, HD:2 * HD],
                                 xq_sb[1][:, sl], start=False, stop=True)
                nc.vector.tensor_scalar_add(q_sb[:, sl], qp[:], bq_sb[:])

        def kv_proj(t):
            kv_proj_mult(t, 0)
            kv_proj_mult(t, 1)

        def q_proj(t):
            q_proj_mult(t, 0)
            q_proj_mult(t, 1)

        def v_transpose(g):  # v cols [1024g, 1024g+1024) -> va blocks 8g..
            nc.sync.dma_start_transpose(
                out=va_v[:, 8 * g:8 * (g + 1), 0:HD],
                in_=v_sb[HD:P, g * IC:(g + 1) * IC])

        def va_gate():
            # in-place +0.0 on va block 0: exact, and makes the first AV
            # (hence the whole in-order PE stream) wait for the gate DMA
            nc.vector.tensor_scalar_add(va_sb[:, 0:P], va_sb[:, 0:P],
                                        gate_sb[:])

        # Interleave schedule: (i, j) -> work emitted before block (i, j).
        pre = {}

        def sched(i, j, fn):
            pre.setdefault((i, j), []).append(fn)

        for c in range(4, 8):
            jj = 4 * c - 6  # kv chunk c metered at j = 10, 14, 18, 22
            sched(0, jj, lambda c=c: kv_proj_mult(c, 0))
            sched(0, jj + 1, lambda c=c: kv_proj_mult(c, 1))
        sched(0, 16, lambda: v_transpose(2))
        sched(0, 24, lambda: v_transpose(3))
        sched(0, 26, lambda: q_proj_mult(2, 0))
        sched(0, 27, lambda: q_proj_mult(2, 1))
        sched(0, 28, lambda: q_proj_mult(3, 0))
        sched(0, 29, lambda: q_proj_mult(3, 1))
        for i, t0 in ((1, 4), (2, 6)):
            for dt in range(2):
                sched(i, 12 + 4 * dt, lambda t=t0 + dt: q_proj_mult(t, 0))
                sched(i, 13 + 4 * dt, lambda t=t0 + dt: q_proj_mult(t, 1))

        # ---- epilogue pieces ----
        pend = [None] * NI

        def drain(i, av, final=False):
            outt = fpool.tile([HD, IC], BF16, name="outt")
            nc.vector.tensor_copy(outt[:], av[0:HD, :])
            den = fpool.tile([1, IC], F32, name="den")
            nc.vector.tensor_copy(den[:], av[HD:HD + 1, :])
            nc.gpsimd.dma_start(yden[:, i * IC:(i + 1) * IC], den[:])
            pend[i] = outt

        def out_proj_step(i, s, final=False):
            outt = pend[i]
            oh, h = divmod(s, 2)
            yp = pp.tile([P, 512], F32, tag="s", bufs=2, name="yp")
            nc.tensor.matmul(yp[:], wo_sb[:, oh * P:(oh + 1) * P],
                             outt[:, h * 512:(h + 1) * 512],
                             start=True, stop=True)
            ys = fpool.tile([P, 512], BF16, name="ys", tag="ys", bufs=4)
            if final and s >= 2:
                nc.scalar.activation(ys[:], yp[:],
                                     mybir.ActivationFunctionType.Copy)
            else:
                nc.vector.tensor_copy(ys[:], yp[:])
            nc.sync.dma_start(
                y[oh][:, i * IC + h * 512:i * IC + (h + 1) * 512], ys[:])

        # ---- attention ----
        def qk_exp(i, j):
            jb = slice(j * P, (j + 1) * P)
            st = pp.tile([P, IC], F32, tag="s", bufs=2, name="st")
            for h in range(2):
                isl = slice(i * IC + h * 512, i * IC + (h + 1) * 512)
                nc.tensor.matmul(st[:, h * 512:(h + 1) * 512],
                                 k_sb[:, jb], q_sb[:, isl],
                                 start=True, stop=True)
            et = epool.tile([P, IC], BF16, name="et")
            nc.scalar.activation(et[:], st[:],
                                 mybir.ActivationFunctionType.Exp,
                                 bias=ebias_sb[:])
            return et

        def av_mm(av, j, et, start, stop):
            for h in range(2):
                nc.tensor.matmul(av[:, h * 512:(h + 1) * 512],
                                 va_v[:, j, :], et[:, h * 512:(h + 1) * 512],
                                 start=start, stop=stop)

        # pre-loop head (cold clock, DMA-paced)
        kv_proj(0)
        kv_proj(1)
        q_proj(0)
        q_proj(1)
        v_transpose(0)
        kv_proj(2)
        kv_proj(3)
        v_transpose(1)
        va_gate()

        for i in range(NI):
            av = pp.tile([P, IC], F32, tag="av", bufs=2, name="av")
            bank = [qk_exp(0, j) for j in range(PRE)] if i == 0 else []
            for j in range(NJ):
                for fn in pre.get((i, j), ()):
                    fn()
                if i > 0 and j == 8:
                    for s in range(4):
                        out_proj_step(i - 1, s)
                et = bank[j] if i == 0 and j < PRE else qk_exp(i, j)
                av_mm(av, j, et, start=(j == 0), stop=(j == NJ - 1))
            drain(i, av)
        for s in range(4):
            out_proj_step(NI - 1, s, final=True)


def build():
    nc = bacc.Bacc("TRN2", target_bir_lowering=False, debug=False,
                   enable_asserts=False)
    with tile.TileContext(nc) as tc:
        _emit(tc)
    nc.compile()
    return nc


_NC_CACHE = []


def _get_nc():
    if not _NC_CACHE:
        _NC_CACHE.append(build())
    return _NC_CACHE[0]


def make_in_maps(x_q, x_kv, wq, bq, wk, bk, wv, bv, wo, bo):
    bf = ml_dtypes.bfloat16
    in_maps = []
    bo_effs = []
    for c in range(NCORES):
        b, n = divmod(c, NH)
        hs = slice(n * HD, (n + 1) * HD)
        wq_h = wq[hs].astype(np.float64) * SCALE
        bo_eff = wo[:, hs].astype(np.float64) @ bv[hs].astype(np.float64)
        if n == 0:
            bo_eff = bo_eff + bo.astype(np.float64)
        bo_effs.append(bo_eff.astype(np.float32))
        in_maps.append({
            "xq": np.ascontiguousarray(
                x_q[b].reshape(C, S).reshape(2, P, S)).astype(bf),
            "xkv": np.ascontiguousarray(
                x_kv[b].reshape(C, S).reshape(2, P, S)).astype(bf),
            "wqT": np.ascontiguousarray(wq_h.T.reshape(2, P, HD)).astype(bf),
            "wkvT": np.ascontiguousarray(
                np.concatenate([wk[hs].T, wv[hs].T], axis=1)
                .reshape(2, P, P)).astype(bf),
            "woT": np.ascontiguousarray(wo[:, hs].T).astype(bf),
            "bq": (bq[hs].astype(np.float64) * SCALE
                   ).astype(np.float32).reshape(HD, 1),
            "gate": np.zeros((P, 1), np.float32),
        })
    return in_maps, bo_effs


def assemble_output(results, bo_effs):
    # y_core is the unnormalized head partial (bf16, scaled by e^-shift);
    # divide by the (identically scaled) denominator and add the
    # host-folded bias here.
    y = np.zeros((B, C, S), np.float32)
    for c in range(NCORES):
        b = c // NH
        den = results[c]["yden"].reshape(1, S).astype(np.float32)
        y[b] += results[c]["y"].astype(np.float32).reshape(C, S) / den \
            + bo_effs[c].reshape(C, 1)
    return y.reshape(B, C, HGT, WID)


def kernel(**inputs):
    nc = _get_nc()
    in_maps, bo_effs = make_in_maps(**inputs)
    res = run_bass_kernel_spmd(nc, in_maps, list(range(NCORES)))
    return assemble_output(res.results, bo_effs)


if __name__ == "__main__":
    nc = build()
    print("built + compiled ok")


# revision 24
# speedup vs baseline: 1.3345x; 1.2949x over previous
"""Cross-attention kernel for Trainium2, sharded over 8 NeuronCores.

Problem (per reference):
  q = wq @ x_q + bq ; k = wk @ x_kv + bk ; v = wv @ x_kv + bv   (1x1 convs)
  per head: attn = softmax(q^T k / sqrt(hd)) ; out = attn @ v^T
  y = wo @ out + bo

Sharding: core c -> (batch b = c // 4, head n = c % 4). Each core runs one
head's full attention and produces the partial output projection
y_part = wo[:, head] @ out_head; the host sums the 4 head partials per batch.

Device-side simplifications (all mathematically exact):
  * bk drops out entirely (per-query constant shift cancels in softmax).
  * bv folds into the output bias on the host (sum_j softmax_ij = 1).
  * scale 1/8 folds into wq/bq on the host.
  * exp computes e^(s - 2.5): the constant shift cancels in the deferred
    host-side normalization and keeps e^s under fp8-e4m3's 448 max.
  * softmax denominator comes from a ones-column appended to v^T in the AV
    matmul.
  * normalization is deferred past the output projection to the host:
    the device ships y_un (bf16) plus per-pixel denominators (f32); the
    host computes y_un / den + bias.

Performance structure (from NTFF traces): the loop is bound by the scalar
engine's exp ([128,1024] PSUM->SBUF sustains ~1.07us back-to-back), but the
chip enforces an activity power cap: sustained PE-array duty above ~80% of
2.4GHz trips a 50%-utilization throttle (observed: 91% duty -> throttled,
77% -> clean). The kernel therefore (a) software-pipelines QK(j) -> exp(j)
-> AV(j-2..j-3) with the logit tile st triple-buffered in PSUM (3x2 banks
+ 2 for the AV accumulator = all 8) so consecutive exps never have PE work
on their dependency path, and (b) runs HALF the AV block-pairs as fp8-e4m3
DoubleRow matmuls (two j-blocks contracted per pass, halving those AV
cycles) to hold PE duty at ~77%, under the cap. fp8 on post-softmax
weights/values is accuracy-safe (~1.4e-2 total rel err vs the 2e-2 gate).
The scalar engine runs nothing but exps; inputs stream in 512-col chunks
(gpsimd ring: weights + x_kv + back-half x_q; sync ring: front x_q, the
four v^T chunk transposes, y outputs) with projections and transposes
interleaved into the first i-chunk."""

import numpy as np
import ml_dtypes

import concourse.bacc as bacc
import concourse.mybir as mybir
import concourse.tile as tile
from concourse.bass_utils import run_bass_kernel_spmd

F32 = mybir.dt.float32
BF16 = mybir.dt.bfloat16
F8 = mybir.dt.float8e4

B, C, HGT, WID = 2, 256, 64, 64
S = HGT * WID  # 4096 pixels
NH, HD = 4, 64
NCORES = 8
P = 128
IC = 1024  # i-chunk width (2 PSUM banks)
NI = S // IC  # 4
NJ = S // P  # 32 j-blocks
SCALE = HD ** -0.5
EXP_SHIFT = 2.5  # exp(s - shift): cancels in host normalization


def _fp8_pair(p):
    """Which global block-pairs run the AV in fp8 DoubleRow."""
    return p % 2 == 0


def _emit(tc):
    nc = tc.nc
    xq = nc.dram_tensor("xq", [2, P, S], BF16, kind="ExternalInput").ap()
    xkv = nc.dram_tensor("xkv", [2, P, S], BF16, kind="ExternalInput").ap()
    wqT = nc.dram_tensor("wqT", [2, P, HD], BF16, kind="ExternalInput").ap()
    wkvT = nc.dram_tensor("wkvT", [2, P, P], BF16, kind="ExternalInput").ap()
    woT = nc.dram_tensor("woT", [HD, C], BF16, kind="ExternalInput").ap()
    bq = nc.dram_tensor("bq", [HD, 1], F32, kind="ExternalInput").ap()
    gate = nc.dram_tensor("gate", [P, 1], F32, kind="ExternalInput").ap()
    y = nc.dram_tensor("y", [2, P, S], BF16, kind="ExternalOutput").ap()
    yden = nc.dram_tensor("yden", [1, S], F32, kind="ExternalOutput").ap()

    with (
        tc.tile_pool(name="const", bufs=1) as cpool,
        tc.tile_pool(name="xp", bufs=1) as xpool,
        tc.tile_pool(name="qkv", bufs=1) as qpool,
        tc.tile_pool(name="es", bufs=1) as epool,
        tc.tile_pool(name="epi", bufs=2) as fpool,
        tc.tile_pool(name="ps", bufs=1, space="PSUM") as pp,
    ):
        # ---- exp bias + table load ASAP (first DVE/ACT work) ----
        ebias_sb = cpool.tile([P, 1], F32)
        nc.vector.memset(ebias_sb[:], -EXP_SHIFT)
        warm_sb = cpool.tile([P, 1], BF16)
        nc.scalar.activation(warm_sb[:], ebias_sb[:],
                             mybir.ActivationFunctionType.Exp,
                             bias=ebias_sb[:])

        # ---- weights + x_kv chunks on the gpsimd (SWDGE) ring ----
        wq_sb = cpool.tile([P, 2 * HD], BF16)
        wkv_sb = cpool.tile([P, 2 * P], BF16)
        wo_sb = cpool.tile([HD, C], BF16)
        bq_sb = cpool.tile([HD, 1], F32)
        xq_sb = [xpool.tile([P, S], BF16, tag=f"xq{i}", name=f"xq_sb{i}")
                 for i in range(2)]
        xkv_sb = [xpool.tile([P, S], BF16, tag=f"xkv{i}", name=f"xkv_sb{i}")
                  for i in range(2)]

        KCH = 512  # x_kv arrival chunk (one kv projection's worth)

        def xkv_chunk(c):
            sl = slice(c * KCH, (c + 1) * KCH)
            nc.gpsimd.dma_start(xkv_sb[0][:, sl], xkv[0][:, sl])
            nc.gpsimd.dma_start(xkv_sb[1][:, sl], xkv[1][:, sl])

        def xq_quarter(eng, qt):
            sl = slice(qt * IC, (qt + 1) * IC)
            eng.dma_start(xq_sb[0][:, sl], xq[0][:, sl])
            eng.dma_start(xq_sb[1][:, sl], xq[1][:, sl])

        gate_sb = cpool.tile([P, 1], F32)
        dly_sb = cpool.tile([P, IC], BF16)  # delay ballast for the gate

        # Front of the input stream on the sync ring (cheap 60ns issues):
        # x_kv chunks 0-3 and the front half of x_q, in consumption order.
        # The sync ring then carries the v^T transposes + y outputs.
        for c in range(2):
            sl = slice(c * KCH, (c + 1) * KCH)
            nc.sync.dma_start(xkv_sb[0][:, sl], xkv[0][:, sl])
            nc.sync.dma_start(xkv_sb[1][:, sl], xkv[1][:, sl])
        xq_quarter(nc.sync, 0)
        for c in range(2, 4):
            sl = slice(c * KCH, (c + 1) * KCH)
            nc.sync.dma_start(xkv_sb[0][:, sl], xkv[0][:, sl])
            nc.sync.dma_start(xkv_sb[1][:, sl], xkv[1][:, sl])
        xq_quarter(nc.sync, 1)

        # Back of the stream + the grant gate on the gpsimd (SWDGE) ring.
        # Ballast DMAs delay the gate so it lands right after the banked
        # phase exhausts -- producing the one quiet window that flips the
        # PE clock to 2.4GHz.
        nc.gpsimd.dma_start(wkv_sb[:, 0:P], wkvT[0])
        nc.gpsimd.dma_start(wkv_sb[:, P:2 * P], wkvT[1])
        nc.gpsimd.dma_start(wq_sb[:, 0:HD], wqT[0])
        nc.gpsimd.dma_start(wq_sb[:, HD:2 * HD], wqT[1])
        nc.gpsimd.dma_start(bq_sb[:], bq)
        nc.gpsimd.dma_start(wo_sb[:], woT)
        for c in range(4, 8):
            xkv_chunk(c)
        xq_quarter(nc.gpsimd, 2)
        xq_quarter(nc.gpsimd, 3)
        for r in range(3):
            nc.gpsimd.dma_start(dly_sb[:], xkv[0][:, r * IC:(r + 1) * IC])
        nc.gpsimd.dma_start(gate_sb[:], gate)

        # ---- persistent SBUF tensors ----
        # q/k kept at their true 64 partitions: matmul contracts over 64.
        q_sb = qpool.tile([HD, S], BF16)
        k_sb = qpool.tile([HD, S], BF16)
        v_sb = qpool.tile([P, S], BF16)  # rows 64:128 hold v
        # v^T blocks: [j-part, (block, 128)]; cols 0:64 = v^T (transpose
        # target), col 64 = ones (denominator row), rest zero. va8 is the
        # fp8 copy used by the DoubleRow AV pairs.
        va_sb = qpool.tile([P, NJ * P], BF16)
        va_v = va_sb.rearrange("p (j c) -> p j c", c=P)
        va8_sb = qpool.tile([P, NJ * P], F8)
        va8_v = va8_sb.rearrange("p (j c) -> p j c", c=P)
        nc.vector.memset(va_sb[:], 0.0)
        nc.vector.memset(va_v[:, :, HD:HD + 1], 1.0)

        # Projection pieces. In-loop projections are emitted one matmul at
        # a time (with an ACT pacing bubble on those blocks) so the
        # post-grant PE duty never spikes past the chip's activity power
        # cap (~77% sustained trips a 50% throttle).
        proj_psum = {}

        def kv_proj_mult(t, half):
            sl = slice(t * 512, (t + 1) * 512)
            if half == 0:
                proj_psum[("kv", t)] = pp.tile([P, 512], F32, tag="s",
                                               bufs=3, name="kvp")
                nc.tensor.matmul(proj_psum[("kv", t)][:], wkv_sb[:, 0:P],
                                 xkv_sb[0][:, sl], start=True, stop=False)
            else:
                kvp = proj_psum.pop(("kv", t))
                nc.tensor.matmul(kvp[:], wkv_sb[:, P:2 * P],
                                 xkv_sb[1][:, sl], start=False, stop=True)
                nc.vector.tensor_copy(k_sb[:, sl], kvp[0:HD, :])
                nc.vector.tensor_copy(v_sb[HD:P, sl], kvp[HD:P, :])

        def q_proj_mult(t, half):
            sl = slice(t * 512, (t + 1) * 512)
            if half == 0:
                proj_psum[("q", t)] = pp.tile([HD, 512], F32, tag="s",
                                              bufs=3, name="qp")
                nc.tensor.matmul(proj_psum[("q", t)][:], wq_sb[:, 0:HD],
                                 xq_sb[0][:, sl], start=True, stop=False)
            else:
                qp = proj_psum.pop(("q", t))
                nc.tensor.matmul(qp[:], wq_sb[:, HD:2 * HD],
                                 xq_sb[1][:, sl], start=False, stop=True)
                nc.vector.tensor_scalar_add(q_sb[:, sl], qp[:], bq_sb[:])

        def kv_proj(t):
            kv_proj_mult(t, 0)
            kv_proj_mult(t, 1)

        def q_proj(t):
            q_proj_mult(t, 0)
            q_proj_mult(t, 1)

        def v_transpose(g):  # v cols [1024g, 1024g+1024) -> va blocks 8g..
            nc.sync.dma_start_transpose(
                out=va_v[:, 8 * g:8 * (g + 1), 0:HD],
                in_=v_sb[HD:P, g * IC:(g + 1) * IC])

        def va_cast(g, gated=False):
            # fp8 copy for the DoubleRow pairs (picks up ones column too).
            # Group 0 adds the gate zeros (exact): the first AV pair -- and
            # the whole in-order PE stream behind it -- then waits for the
            # gate DMA, creating the clock-grant quiet window.
            gsl = slice(8 * g * P, 8 * (g + 1) * P)
            if gated:
                nc.vector.tensor_scalar_add(va8_sb[:, gsl], va_sb[:, gsl],
                                            gate_sb[:])
            else:
                nc.vector.tensor_copy(va8_sb[:, gsl], va_sb[:, gsl])

        # Interleave schedule: (i, j) -> work emitted before QK(i, j).
        # Transposes/casts are EMITTED before their first consumer AV
        # (otherwise the framework orders the write after those reads,
        # which would then see the memset zeros). The front of the input
        # (kv chunks 0-3, q chunks 0-1, transposes 0-1) is emitted before
        # the loop; the rest is metered one matmul per block.
        pre = {}

        def sched(i, j, fn):
            pre.setdefault((i, j), []).append(fn)

        for c in range(4, 8):
            jj = 4 * c - 9  # 7, 11, 15, 19 (block 4c first needs chunk c)
            sched(0, jj, lambda c=c: kv_proj_mult(c, 0))
            sched(0, jj + 1, lambda c=c: kv_proj_mult(c, 1))
        sched(0, 13, lambda: v_transpose(2))
        sched(0, 14, lambda: va_cast(2))
        sched(0, 21, lambda: v_transpose(3))
        sched(0, 22, lambda: va_cast(3))
        sched(0, 24, lambda: q_proj_mult(2, 0))
        sched(0, 25, lambda: q_proj_mult(2, 1))
        sched(0, 26, lambda: q_proj_mult(3, 0))
        sched(0, 27, lambda: q_proj_mult(3, 1))
        for i, t0 in ((1, 4), (2, 6)):
            for dt in range(2):
                sched(i, 12 + 4 * dt, lambda t=t0 + dt: q_proj_mult(t, 0))
                sched(i, 13 + 4 * dt, lambda t=t0 + dt: q_proj_mult(t, 1))

        # ---- epilogue pieces ----
        pend = [None] * NI  # per chunk: unnormalized out^T awaiting out-proj

        def drain(i, av):
            # move the (unnormalized) attention output + denominators out
            # of PSUM so the av banks free up; DVE only.
            outt = fpool.tile([HD, IC], BF16, name="outt")
            nc.vector.tensor_copy(outt[:], av[0:HD, :])
            den = fpool.tile([1, IC], F32, name="den")
            nc.vector.tensor_copy(den[:], av[HD:HD + 1, :])
            nc.gpsimd.dma_start(yden[:, i * IC:(i + 1) * IC], den[:])
            pend[i] = outt

        def out_proj_step(i, s, final=False):
            # one quarter of chunk i's output projection
            outt = pend[i]
            oh, h = divmod(s, 2)
            yp = pp.tile([P, 512], F32, tag="s", bufs=3, name="yp")
            nc.tensor.matmul(yp[:], wo_sb[:, oh * P:(oh + 1) * P],
                             outt[:, h * 512:(h + 1) * 512],
                             start=True, stop=True)
            ys = fpool.tile([P, 512], BF16, name="ys", tag="ys", bufs=4)
            if final and s >= 2:
                # ACT is idle after the last exp: split the drains
                nc.scalar.activation(ys[:], yp[:],
                                     mybir.ActivationFunctionType.Copy)
            else:
                nc.vector.tensor_copy(ys[:], yp[:])
            nc.sync.dma_start(
                y[oh][:, i * IC + h * 512:i * IC + (h + 1) * 512], ys[:])

        # ---- the attention loop ----
        # Per global block b = i*NJ + j: QK(b) -> exp(b) -> AV(b-3, b-2)
        # at odd b. st triple-buffered: back-to-back exps never wait on PE.
        av_tiles = [None] * NI
        et8 = {}   # pair index -> [P, 2*IC] fp8 tile
        et16 = {}  # block index -> [P, IC] bf16 tile

        def av_pair(b0):  # blocks b0, b0+1 (same chunk: chunks 32-aligned)
            i = b0 // NJ
            j0 = b0 % NJ
            p = b0 // 2
            if av_tiles[i] is None:
                av_tiles[i] = pp.tile([P, IC], F32, tag="av", bufs=1,
                                      name="av")
            av = av_tiles[i]
            start = j0 == 0
            stop = j0 == NJ - 2
            # start/stop are per accumulation REGION (each 512-col bank)
            if _fp8_pair(p):
                e8 = et8.pop(p)
                e8v = e8.rearrange("p (t i) -> p t i", t=2)
                for h in range(2):
                    nc.tensor.matmul(
                        av[:, h * 512:(h + 1) * 512],
                        va8_v[:, j0:j0 + 2, :],
                        e8v[:, :, h * 512:(h + 1) * 512],
                        start=start, stop=stop,
                        perf_mode=mybir.MatmulPerfMode.DoubleRow)
            else:
                for jj in (b0, b0 + 1):
                    et = et16.pop(jj)
                    for h in range(2):
                        nc.tensor.matmul(
                            av[:, h * 512:(h + 1) * 512],
                            va_v[:, jj % NJ, :],
                            et[:, h * 512:(h + 1) * 512],
                            start=start and jj == b0,
                            stop=stop and jj == b0 + 1)
            if stop:
                drain(i, av)
                av_tiles[i] = None

        # ---- pre-loop head: front projections at the cold clock ----
        kv_proj(0)
        kv_proj(1)
        q_proj(0)
        q_proj(1)
        v_transpose(0)
        kv_proj(2)
        kv_proj(3)
        v_transpose(1)
        va_cast(0, gated=True)
        va_cast(1)

        NB = NI * NJ
        for b in range(NB):
            i, j = divmod(b, NJ)
            for fn in pre.get((i, j), ()):
                fn()
            if i > 0 and 2 <= j <= 8 and j % 2 == 0:
                # one quarter of the previous chunk's output projection
                out_proj_step(i - 1, j // 2 - 1)
            # QK + exp for block b
            jb = slice(j * P, (j + 1) * P)
            st = pp.tile([P, IC], F32, tag="s", bufs=3, name="st")
            for h in range(2):
                isl = slice(i * IC + h * 512, i * IC + (h + 1) * 512)
                nc.tensor.matmul(st[:, h * 512:(h + 1) * 512],
                                 k_sb[:, jb], q_sb[:, isl],
                                 start=True, stop=True)
            p = b // 2
            if _fp8_pair(p):
                if b % 2 == 0:
                    et8[p] = epool.tile([P, 2 * IC], F8, name="et8",
                                        tag="et8", bufs=3)
                nc.scalar.activation(et8[p][:, (b % 2) * IC:(b % 2 + 1) * IC],
                                     st[:],
                                     mybir.ActivationFunctionType.Exp,
                                     bias=ebias_sb[:])
            else:
                et16[b] = epool.tile([P, IC], BF16, name="et16",
                                     tag="et16", bufs=6)
                nc.scalar.activation(et16[b][:], st[:],
                                     mybir.ActivationFunctionType.Exp,
                                     bias=ebias_sb[:])
            if b >= 3 and b % 2 == 1:
                av_pair(b - 3)
        av_pair(NB - 2)
        for s in range(4):
            out_proj_step(NI - 1, s, final=True)


def build():
    nc = bacc.Bacc("TRN2", target_bir_lowering=False, debug=False,
                   enable_asserts=False)
    with tile.TileContext(nc) as tc:
        _emit(tc)
    nc.compile()
    return nc


_NC_CACHE = []


def _get_nc():
    if not _NC_CACHE:
        _NC_CACHE.append(build())
    return _NC_CACHE[0]


def make_in_maps(x_q, x_kv, wq, bq, wk, bk, wv, bv, wo, bo):
    bf = ml_dtypes.bfloat16
    in_maps = []
    bo_effs = []
    for c in range(NCORES):
        b, n = divmod(c, NH)
        hs = slice(n * HD, (n + 1) * HD)
        wq_h = wq[hs].astype(np.float64) * SCALE
        bo_eff = wo[:, hs].astype(np.float64) @ bv[hs].astype(np.float64)
        if n == 0:
            bo_eff = bo_eff + bo.astype(np.float64)
        bo_effs.append(bo_eff.astype(np.float32))
        in_maps.append({
            "xq": np.ascontiguousarray(
                x_q[b].reshape(C, S).reshape(2, P, S)).astype(bf),
            "xkv": np.ascontiguousarray(
                x_kv[b].reshape(C, S).reshape(2, P, S)).astype(bf),
            "wqT": np.ascontiguousarray(wq_h.T.reshape(2, P, HD)).astype(bf),
            "wkvT": np.ascontiguousarray(
                np.concatenate([wk[hs].T, wv[hs].T], axis=1)
                .reshape(2, P, P)).astype(bf),
            "woT": np.ascontiguousarray(wo[:, hs].T).astype(bf),
            "bq": (bq[hs].astype(np.float64) * SCALE
                   ).astype(np.float32).reshape(HD, 1),
            "gate": np.zeros((P, 1), np.float32),
        })
    return in_maps, bo_effs


def assemble_output(results, bo_effs):
    # y_core is the unnormalized head partial (bf16, scaled by e^-shift);
    # divide by the (identically scaled) denominator and add the
    # host-folded bias here.
    y = np.zeros((B, C, S), np.float32)
    for c in range(NCORES):
        b = c // NH
        den = results[c]["yden"].reshape(1, S).astype(np.float32)
        y[b] += results[c]["y"].astype(np.float32).reshape(C, S) / den \
            + bo_effs[c].reshape(C, 1)
    return y.reshape(B, C, HGT, WID)


def kernel(**inputs):
    nc = _get_nc()
    in_maps, bo_effs = make_in_maps(**inputs)
    res = run_bass_kernel_spmd(nc, in_maps, list(range(NCORES)))
    return assemble_output(res.results, bo_effs)


if __name__ == "__main__":
    nc = build()
    print("built + compiled ok")
